# revision 11
# baseline (speedup 1.0000x reference)
"""Trainium2 Bass kernel for the ClusterLoss problem.

Loss = mean-entropy(softmax over K of [T, M, K] logits)            (L1)
       - mean-entropy(softmax over K of batch-mean logits [M, K])  (L2)

T=4096, M=64, K=256 hardcoded. Data-parallel over T across 8 cores.

v2 design (vs v1): the input shard is passed partition-major
([128, 256 tiles, 256 K], so DMA lines are 16 KiB contiguous per
partition) and cast fp32->bf16 *during* the DMA (SWDGE), halving SBUF
traffic and enabling 2x DVE perf mode downstream:
  - gpsimd cast-DMA: x fp32 HBM -> xb bf16 SBUF, MEGA tiles per DMA.
  - ACT: exp. SPLITF tiles per mega run as per-tile activations with
    accum_out -> Z (mode A); the rest run as ONE batched exp (amortizes
    the ~352-cycle ACT instruction overhead) with Z computed by a DVE
    3D tensor_reduce (mode B). SPLITF balances ACT vs DVE busy.
  - DVE: per-tile fused affine_mul_reduce w = xb*e (in-place over e),
    accum_out -> S columns (all bf16 operands -> 2x mode; fp32 accum).
  - PE: 0/1-pattern matmul accumulates per-block sums of xb over T in
    PSUM (for L2's batch-mean logits), moving data bf16.
  - tail: H_sum/partition = sum ln(Z) - sum S/Z batched on the stat
    buffers.
Outputs per core: ent [128,1] (partition-sums of per-row entropies) and
bsum [64,256] (partial sum over this core's T rows). Host reduces those
tiny tensors into the final scalar.

No max-subtraction in the softmax: inputs are standard-normal, |x| < ~6,
so exp(x) is comfortably inside fp32/bf16 range; H = ln(Z) - S/Z is
analytically identical to the reference's log_softmax entropy (applied
to the bf16-rounded logits; the rounding error averages out far below
the 2e-2 gate).
"""

from contextlib import nullcontext

import numpy as np

import concourse.bacc as bacc
import concourse.bass as bass
import concourse.tile as tile
from concourse import mybir
from concourse.bass_utils import run_bass_kernel_spmd

T, M, K = 4096, 64, 256
NCORES = 8
TSH = T // NCORES            # 512 t-rows per core
ROWS = TSH * M               # 32768 (t, m) rows per core
P = 128                      # SBUF partitions per tile
NTILES = ROWS // P           # 256 tiles of [128, 256] per core
MEGA = 16                    # row-tiles per DMA (4 MiB fp32 read)
SPLITF = 5                   # tiles per mega with ACT-accum Z (mode A)
PAIR = 2                     # row-tiles per PE matmul (moving free 512)

FP32 = mybir.dt.float32
BF16 = mybir.dt.bfloat16


def _build_nc(repeat=1, use_pe=True, use_act=True, use_dve=True,
              mega=MEGA, splitf=SPLITF, fuse_s=True, bufs=3):
    nc = bacc.Bacc("TRN2", target_bir_lowering=False, debug=False)
    nmega = NTILES // mega

    x_d = nc.dram_tensor("x", [P, NTILES, K], FP32, kind="ExternalInput")
    w_d = nc.dram_tensor("wpat", [P, M], BF16, kind="ExternalInput")
    ent_d = nc.dram_tensor("ent", [P, 1], FP32, kind="ExternalOutput")
    bsum_d = nc.dram_tensor("bsum", [M, K], FP32, kind="ExternalOutput")

    x = x_d.ap()

    with tile.TileContext(nc) as tc:
        with (
            tc.tile_pool(name="xin", bufs=bufs) as xpool,
            tc.tile_pool(name="exp", bufs=bufs) as epool,
            tc.tile_pool(name="fold", bufs=bufs) as fold,
            tc.tile_pool(name="fold2", bufs=bufs) as fold2,
            tc.tile_pool(name="stats", bufs=1) as stats,
            tc.tile_pool(name="small", bufs=1) as small,
            tc.tile_pool(name="psum", bufs=1, space="PSUM") as psum,
            tc.For_i(0, repeat, 1) if repeat > 1 else nullcontext(),
        ):
            wp = small.tile([P, M], BF16)
            nc.sync.dma_start(out=wp, in_=w_d.ap())

            f_per_mega = splitf if (use_act and use_dve) else (mega if use_act else 0)
            n_a = nmega * f_per_mega
            n_b = NTILES - n_a
            zbuf_a = stats.tile([P, max(n_a, 1)], FP32)   # ACT-accum Z
            zbuf_b = stats.tile([P, max(n_b, 1)], BF16)   # DVE-reduced Z
            sbuf_a = stats.tile([P, max(n_a, 1)], FP32)   # S accums, mode A
            sbuf_b = stats.tile([P, max(n_b, 1)], FP32)   # S accums, mode B
            bs_ps = psum.tile([M, PAIR, K], FP32)         # block sums (2 halves)

            ia = ib = 0
            for mg in range(nmega):
                xb = xpool.tile([P, mega, K], BF16)
                nc.gpsimd.dma_start(
                    out=xb, in_=x[:, mg * mega:(mg + 1) * mega, :])
                et = epool.tile([P, mega, K], BF16)
                f = f_per_mega
                # mode A tiles: fused exp + Z on ACT
                for j in range(f):
                    if use_act:
                        nc.scalar.activation(
                            out=et[:, j, :], in_=xb[:, j, :],
                            func=mybir.ActivationFunctionType.Exp,
                            accum_out=zbuf_a[:, ia:ia + 1],
                        )
                    if use_dve:
                        if fuse_s:
                            nc.vector.affine_mul_reduce(
                                out=et[:, j, :], accum_out=sbuf_a[:, ia:ia + 1],
                                in0=xb[:, j, :], in1=et[:, j, :],
                                scale=1.0, bias=0.0,
                            )
                    ia += 1
                # mode B tiles: one batched exp; Z via a bf16 2x pairwise
                # fold (halves the 1x-only tensor_reduce payload) + 3D reduce
                if f < mega:
                    if use_act:
                        nc.scalar.activation(
                            out=et[:, f:, :], in_=xb[:, f:, :],
                            func=mybir.ActivationFunctionType.Exp,
                        )
                    if use_dve:
                        nb = mega - f
                        zf1 = fold.tile([P, nb, K // 2], BF16)
                        zf2 = fold2.tile([P, nb, K // 4], BF16)
                        with nc.allow_low_precision(
                            reason="Z in bf16 for 2x DVE; 0.4% on ln(Z) "
                                   "averages out over 262k rows"
                        ):
                            # fold1 on GPSIMD (otherwise idle), fold2 on DVE
                            # at 2x, then the 1x-only reduce sees K/4 elems
                            nc.gpsimd.tensor_add(
                                zf1, et[:, f:, 0:K // 2], et[:, f:, K // 2:K])
                            nc.vector.tensor_add(
                                zf2, zf1[:, :, 0:K // 4], zf1[:, :, K // 4:])
                            nc.vector.tensor_reduce(
                                out=zbuf_b[:, ib:ib + nb],
                                in_=zf2,
                                axis=mybir.AxisListType.X,
                                op=mybir.AluOpType.add,
                            )
                        for j in range(f, mega):
                            if fuse_s:
                                nc.vector.affine_mul_reduce(
                                    out=et[:, j, :],
                                    accum_out=sbuf_b[:, ib:ib + 1],
                                    in0=xb[:, j, :], in1=et[:, j, :],
                                    scale=1.0, bias=0.0,
                                )
                            ib += 1
                    else:
                        ib += mega - f
                if use_pe:
                    for j in range(mega // PAIR):
                        g = mg * (mega // PAIR) + j
                        nc.tensor.matmul(
                            bs_ps,
                            wp,
                            xb[:, j * PAIR:(j + 1) * PAIR, :],
                            start=(g == 0),
                            stop=(g == nmega * (mega // PAIR) - 1),
                        )

            # ---- tail: batched entropy math over the stat buffers ----
            parts = []
            for idx, (zb, sb, n) in enumerate(
                ((zbuf_a, sbuf_a, n_a), (zbuf_b, sbuf_b, n_b))
            ):
                if n == 0:
                    continue
                if not (use_act and use_dve):
                    nc.vector.memset(zb, 1.0)
                    nc.vector.memset(sb, 0.0)
                logz = stats.tile([P, n], FP32, tag=f"logz{idx}")
                lsum = small.tile([P, 1], FP32, tag=f"lsum{idx}")
                nc.scalar.activation(
                    out=logz, in_=zb,
                    func=mybir.ActivationFunctionType.Ln,
                    accum_out=lsum,
                )
                rz = stats.tile([P, n], FP32, tag=f"rz{idx}")
                nc.vector.reciprocal(out=rz, in_=zb)
                szsum = small.tile([P, 1], FP32, tag=f"szsum{idx}")
                nc.vector.affine_mul_reduce(
                    out=rz, accum_out=szsum,
                    in0=sb, in1=rz,
                    scale=1.0, bias=0.0,
                )
                part = small.tile([P, 1], FP32, tag=f"part{idx}")
                nc.vector.tensor_sub(part, lsum, szsum)
                parts.append(part)
            ent_sb = small.tile([P, 1], FP32)
            if len(parts) == 2:
                nc.vector.tensor_add(ent_sb, parts[0], parts[1])
            else:
                nc.vector.tensor_copy(out=ent_sb, in_=parts[0])
            nc.sync.dma_start(out=ent_d.ap(), in_=ent_sb)

            bsum_sb = small.tile([M, K], FP32)
            if use_pe:
                nc.scalar.copy(bsum_sb, bs_ps[:, 0, :])
                nc.vector.tensor_add(bsum_sb, bsum_sb, bs_ps[:, 1, :])
            else:
                nc.vector.memset(bsum_sb, 0.0)
            nc.sync.dma_start(out=bsum_d.ap(), in_=bsum_sb)

    nc.compile()
    return nc


_NC_CACHE = []


def _get_nc():
    if not _NC_CACHE:
        _NC_CACHE.append(_build_nc())
    return _NC_CACHE[0]


def _wpat():
    wp = np.zeros((P, M), mybir.dt.np(BF16))
    wp[np.arange(P), np.arange(P) % M] = 1.0
    return wp


def _shard_host(xf):
    """[T, M*K] fp32 -> per-core partition-major [NCORES, P, NTILES, K]."""
    s = xf.reshape(NCORES, NTILES, P, K).transpose(0, 2, 1, 3)
    return np.ascontiguousarray(s)


def kernel(block_feats, **kw):
    assert int(kw.get("M", M)) == M
    xf = np.asarray(block_feats, dtype=np.float32)
    assert xf.shape == (T, M * K)
    shards = _shard_host(xf)

    nc = _get_nc()
    wp = _wpat()
    in_maps = [{"x": shards[i], "wpat": wp} for i in range(NCORES)]
    res = run_bass_kernel_spmd(nc, in_maps, core_ids=list(range(NCORES))).results

    ent_total = sum(float(r["ent"].sum(dtype=np.float64)) for r in res)
    L1 = ent_total / (T * M)

    bs = np.zeros((M, K), np.float64)
    for r in res:
        bs += r["bsum"]
    bm = bs / T
    z = bm - bm.max(axis=-1, keepdims=True)
    e = np.exp(z)
    Z = e.sum(axis=-1, keepdims=True)
    logp = z - np.log(Z)
    H = -(np.exp(logp) * logp).sum(axis=-1)
    L2 = -H.mean()

    return np.asarray(L1 + L2, dtype=np.float32)
